# revision 6
# baseline (speedup 1.0000x reference)
"""DeltaDequantization Trainium2 kernel (8-core SPMD, pure data parallel over batch).

Math (per batch element b, chunks c of 32 steps):
    scale_c = (1/32) * sum_{s,n} x[b,c,s,n] * cs[n]          (independent of carry!)
    S_c     = prod_{c'<c} scale_c'          (exclusive cumprod)
    y[b,t]  = sum_n x[b,t,n] * qb[n]
    m_c     = (1/32) * sum_{s in c} y[b,t]
    pred_c  = sum_{c'<c} S_c' * m_c'        (exclusive cumsum)
    out[b,t]= pred_c(t) + S_c(t) * y[b,t]

Strip-pipelined (strip = 64 timesteps = one 1 MiB SWDGE cast-load, 32 strips).
Per strip: 16 PE 128x128 transposes -> (t'',n)-on-partition layout, 4
accumulating [128,32]x[128,512] matmuls (qb & cs/32 columns interleaved) into
a rotated 32-row PSUM band, band->SBUF copy (ACT), 4 small [32->128] PE
transposes back to batch-major, y-extract with fused chunk-sum (ACT
activation accum_out), w chunk-sums (DVE multi-axis reduce).  Per span
(4 strips): cumprod/cumsum scan cluster (DVE), per-chunk affine as one ACT
activation (out = y*S_c + pred_c via scale/bias APs), HWDGE store.

Engine budget per 2.49us strip: PE ~2.0, DVE ~1.7, ACT ~1.5, Pool ~0.5.
DVE uses only tensor_tensor/reduce/scan (1-port ops) so SWDGE descriptor
generation is never stalled.  Cross-strip emission shifts (strip-tr at s-1,
w-reduce at s-2) keep the in-order engines from blocking on fresh deps.
"""

import numpy as np

import concourse.bass as bass
import concourse.bacc as bacc
import concourse.tile as tile
from concourse import mybir
from concourse.bass_utils import run_bass_kernel_spmd
from concourse.masks import make_identity

F32 = mybir.dt.float32
BF16 = mybir.dt.bfloat16

B, T, NB = 1024, 2048, 32
NCORES = 8
BS = B // NCORES          # 128 batch rows per core = full partition dim
ADAPT = 32
C = T // ADAPT            # 64 chunks
STRIP_T = 64              # timesteps per strip = one 1 MiB f32 load
NSTRIP = T // STRIP_T     # 32
SF = STRIP_T * NB         # 2048 elements per partition per strip
PREFETCH = 14             # strip loads in flight ahead of compute

_cached_nc = None


def build_kernel():
    nc = bacc.Bacc("TRN2", target_bir_lowering=False, debug=False)

    x_ext = nc.dram_tensor("x", [BS, T * NB], F32, kind="ExternalInput")
    qb_ext = nc.dram_tensor("quant_bins", [NB, 1], F32, kind="ExternalInput")
    cs_ext = nc.dram_tensor("change_scales", [NB, 1], F32, kind="ExternalInput")
    out_ext = nc.dram_tensor("out", [BS, T], F32, kind="ExternalOutput")

    ADD = mybir.AluOpType.add
    MUL = mybir.AluOpType.mult
    BYP = mybir.AluOpType.bypass
    AFCOPY = mybir.ActivationFunctionType.Copy
    AFID = mybir.ActivationFunctionType.Identity

    with tile.TileContext(nc) as tc:
        with (
            tc.tile_pool(name="consts", bufs=1) as consts,
            tc.tile_pool(name="xpool", bufs=16) as xpool,
            tc.tile_pool(name="xtpool", bufs=3) as xtpool,
            tc.tile_pool(name="ywpool", bufs=2) as ywpool,
            tc.tile_pool(name="ypool", bufs=2) as ypool,
            tc.tile_pool(name="accpool", bufs=1) as accpool,
            tc.tile_pool(name="ps_t", bufs=3, space="PSUM") as ps_t,
            tc.tile_pool(name="ps_b", bufs=2, space="PSUM") as ps_b,
            tc.tile_pool(name="ps_s", bufs=3, space="PSUM") as ps_s,
        ):
            # small consts / scan chains (tiny DVE memsets: no Q7 stall)
            zbf = consts.tile([128, 1], BF16)
            inv32 = consts.tile([128, 1], F32)
            nc.vector.memset(zbf[:], 0.0)
            nc.vector.memset(inv32[:], 1.0 / ADAPT)
            S_chain = consts.tile([128, C + 1], F32)
            pred_chain = consts.tile([128, C + 1], F32)
            nc.vector.memset(S_chain[:, 0:1], 1.0)
            nc.vector.memset(pred_chain[:, 0:1], 0.0)
            m_buf = consts.tile([128, C], F32)
            p_buf = consts.tile([128, C], F32)
            tau_buf = consts.tile([128, C], F32)

            # qb/cs staging via HWDGE (keeps the SWDGE ring free)
            qbcs = consts.tile([128, 2], F32)
            for tp in range(4):
                nc.sync.dma_start(out=qbcs[32 * tp:32 * tp + 32, 0:1], in_=qb_ext[:])
                nc.sync.dma_start(out=qbcs[32 * tp:32 * tp + 32, 1:2], in_=cs_ext[:])

            xh = [
                xpool.tile([128, SF], BF16, name="xh", tag="xh")
                for _ in range(NSTRIP)
            ]

            def issue_load(s):
                nc.gpsimd.dma_start(out=xh[s][:], in_=x_ext[:, s * SF:(s + 1) * SF])

            issue_load(0)
            issue_load(1)

            ident_bf = consts.tile([128, 128], BF16)
            make_identity(nc, ident_bf[:])

            for s in range(2, PREFETCH):
                issue_load(s)

            # Four stationary matrices A32_q [128, 32], q = 0..3, built on ACT.
            # Column m = 16*j + 4*q + t''; A32_q[(t', n), m] = delta(t', t'') *
            # (qb[n] if j == 0 else cs[n]/32); zero columns for other q.
            A32 = []
            for q in range(4):
                Aq = consts.tile([128, 32], BF16, tag=f"A32_{q}")
                nc.scalar.memzero(Aq[:])
                for tp in range(4):
                    sl = slice(32 * tp, 32 * tp + 32)
                    nc.scalar.mul(Aq[sl, 4 * q + tp:4 * q + tp + 1], qbcs[sl, 0:1], 1.0)
                    nc.scalar.mul(
                        Aq[sl, 16 + 4 * q + tp:16 + 4 * q + tp + 1],
                        qbcs[sl, 1:2],
                        1.0 / ADAPT,
                    )
                A32.append(Aq)

            out_sb = accpool.tile([128, T], F32)

            # per-strip state carried across the shifted pipeline stages
            bandof = {}   # s -> (psum band tile, row offset r)
            ywof = {}     # s -> sbuf band tile
            slabof = {}   # s -> psum slab tile
            yspan = {}    # span k -> [128, 256] f32 y values
            zb1024 = zbf[:, 0:1].broadcast_to([128, 1024])

            def pe_strip_tr(sm):
                # [32,128] transposes of the yw band back to batch-major.
                # slab free index = 32*blk + 16*j + 4*q + t''
                r = 32 * (sm % 4)
                yw = ywof.pop(sm)
                slab = ps_s.tile([128, 128], BF16)
                slabof[sm] = slab
                for blk2 in range(4):
                    nc.tensor.transpose(
                        slab[:, 32 * blk2:32 * blk2 + 32],
                        yw[r:r + 32, 128 * blk2:128 * (blk2 + 1)],
                        ident_bf[r:r + 32, r:r + 32],
                        tile_position=(r, 0),
                    )

            def act_band(sm):
                r = 32 * (sm % 4)
                band = bandof.pop(sm)
                yw = ywpool.tile([128, 512], BF16)
                ywof[sm] = yw
                nc.scalar.copy(out=yw[r:r + 32, :], in_=band[r:r + 32, :])

            def act_yext(sm):
                # y chunk extract + fused chunk sum (t = 16q + 4blk + t'')
                k = sm // 4
                if sm % 4 == 0:
                    yspan[k] = ypool.tile([128, 4 * STRIP_T], F32, name="ysp", tag="ysp")
                slab = slabof[sm]
                sv = slab[:].rearrange(
                    "p (blk j c q2 t) -> p j c q2 blk t", blk=4, j=2, c=2, q2=2, t=4
                )
                for c in range(2):
                    dst = yspan[k][:, (sm % 4) * STRIP_T + 32 * c:
                                   (sm % 4) * STRIP_T + 32 * (c + 1)]
                    dv = dst.rearrange("p (q2 blk t) -> p q2 blk t", q2=2, blk=4, t=4)
                    nc.scalar.activation(
                        out=dv,
                        in_=sv[:, 0:1, c:c + 1].squeeze(1).squeeze(1),
                        func=AFCOPY,
                        accum_out=m_buf[:, 2 * sm + c:2 * sm + c + 1],
                    )

            def dve_wred(sm):
                slab = slabof.pop(sm)
                sv = slab[:].rearrange(
                    "p (blk j c q2 t) -> p j c q2 blk t", blk=4, j=2, c=2, q2=2, t=4
                )
                nc.vector.tensor_reduce(
                    out=p_buf[:, 2 * sm:2 * sm + 2],
                    in_=sv[:, 1:2].squeeze(1),
                    axis=mybir.AxisListType.XYZ,
                    op=ADD,
                )

            def dve_cluster(k):
                c8 = slice(8 * k, 8 * k + 8)
                nc.vector.tensor_tensor_scan(
                    out=S_chain[:, 8 * k + 1:8 * k + 9],
                    data0=p_buf[:, c8],
                    data1=p_buf[:, c8],
                    initial=S_chain[:, 8 * k:8 * k + 1],
                    op0=MUL,
                    op1=BYP,
                )
                nc.vector.tensor_tensor(
                    out=tau_buf[:, c8], in0=S_chain[:, c8], in1=m_buf[:, c8], op=MUL
                )
                nc.vector.tensor_tensor(
                    out=tau_buf[:, c8], in0=tau_buf[:, c8],
                    in1=inv32[:, 0:1].broadcast_to([128, 8]), op=MUL,
                )
                nc.vector.tensor_tensor_scan(
                    out=pred_chain[:, 8 * k + 1:8 * k + 9],
                    data0=tau_buf[:, c8],
                    data1=tau_buf[:, c8],
                    initial=pred_chain[:, 8 * k:8 * k + 1],
                    op0=ADD,
                    op1=BYP,
                )

            def act_affine(k):
                ysp = yspan.pop(k)
                for c in range(8):
                    cc = 8 * k + c
                    nc.scalar.activation(
                        out=out_sb[:, 32 * cc:32 * cc + 32],
                        in_=ysp[:, 32 * c:32 * c + 32],
                        func=AFID,
                        bias=pred_chain[:, cc:cc + 1],
                        scale=S_chain[:, cc:cc + 1],
                    )

            def sync_store(k):
                t_lo, t_hi = 256 * k, 256 * (k + 1)
                nc.sync.dma_start(out=out_ext[:, t_lo:t_hi], in_=out_sb[:, t_lo:t_hi])

            for s in range(NSTRIP):
                if s + PREFETCH < NSTRIP:
                    issue_load(s + PREFETCH)
                r = 32 * (s % 4)
                x_h = xh[s]

                # PE: 16 transposes -> (t'',n) on partitions, free = b
                xT = xtpool.tile([128, SF], BF16)
                for h in range(2):
                    pst = ps_t.tile([128, 1024], BF16)
                    for k in range(8):
                        blk = 8 * h + k
                        nc.tensor.transpose(
                            pst[:, k * 128:(k + 1) * 128],
                            x_h[:, blk * 128:(blk + 1) * 128],
                            ident_bf[:],
                        )
                    # DVE 2x-bf16 copy as tensor_tensor (1-port; no Q7 stall)
                    nc.vector.tensor_tensor(
                        out=xT[:, h * 1024:(h + 1) * 1024], in0=pst[:],
                        in1=zb1024, op=ADD,
                    )

                # PE: y/w projection into 32-row PSUM band at rotated position
                band = ps_b.tile([128, 512], F32)
                bandof[s] = band
                for q in range(4):
                    nc.tensor.matmul(
                        band[r:r + 32, :],
                        A32[q][:],
                        xT[:, q * 512:(q + 1) * 512],
                        start=(q == 0),
                        stop=(q == 3),
                        tile_position=(0, r),
                    )
                # shifted stages (keep in-order engines from stalling)
                if s >= 1:
                    act_band(s - 1)     # ACT: band(s-1) -> SBUF bf16
                    pe_strip_tr(s - 1)  # PE: 4 small transposes of band(s-1)
                    act_yext(s - 1)     # ACT: y extract + chunk sums
                if s >= 2:
                    dve_wred(s - 2)     # DVE: w chunk sums
                if s >= 5 and s % 4 == 1:
                    k = (s - 5) // 4
                    dve_cluster(k)      # DVE: scans for span k
                    act_affine(k)       # ACT: out = y*S + pred
                    sync_store(k)

            # drain the shifted pipeline (span 7)
            act_band(NSTRIP - 1)
            pe_strip_tr(NSTRIP - 1)
            act_yext(NSTRIP - 1)
            dve_wred(NSTRIP - 2)
            dve_wred(NSTRIP - 1)
            dve_cluster(7)
            act_affine(7)
            sync_store(7)

    nc.compile()
    return nc


def make_in_maps(inputs):
    x = np.ascontiguousarray(inputs["x"], dtype=np.float32)
    qb = np.ascontiguousarray(inputs["quant_bins"], dtype=np.float32).reshape(NB, 1)
    cs = np.ascontiguousarray(inputs["change_scales"], dtype=np.float32).reshape(NB, 1)
    return [
        {
            "x": x[i * BS:(i + 1) * BS].reshape(BS, T * NB),
            "quant_bins": qb,
            "change_scales": cs,
        }
        for i in range(NCORES)
    ]


def gather_out(res):
    out = np.concatenate([res.results[i]["out"] for i in range(NCORES)], axis=0)
    return out.astype(np.float32)


def kernel(x, quant_bins, change_scales):
    global _cached_nc
    if _cached_nc is None:
        _cached_nc = build_kernel()
    nc = _cached_nc

    in_maps = make_in_maps(
        {"x": x, "quant_bins": quant_bins, "change_scales": change_scales}
    )
    res = run_bass_kernel_spmd(nc, in_maps, core_ids=list(range(NCORES)))
    return gather_out(res)


if __name__ == "__main__":
    rng = np.random.default_rng(0)
    x = rng.standard_normal((B, T, NB)).astype(np.float32)
    qb = rng.standard_normal((NB,)).astype(np.float32)
    cs = rng.uniform(0.9, 1.1, (NB, 1)).astype(np.float32)
    out = kernel(x=x, quant_bins=qb, change_scales=cs)
    print("out", out.shape, out.dtype)


# revision 8
# speedup vs baseline: 1.0048x; 1.0048x over previous
"""DeltaDequantization Trainium2 kernel (8-core SPMD, pure data parallel over batch).

Math (per batch element b, chunks c of 32 steps):
    scale_c = (1/32) * sum_{s,n} x[b,c,s,n] * cs[n]          (independent of carry!)
    S_c     = prod_{c'<c} scale_c'          (exclusive cumprod)
    y[b,t]  = sum_n x[b,t,n] * qb[n]
    m_c     = (1/32) * sum_{s in c} y[b,t]
    pred_c  = sum_{c'<c} S_c' * m_c'        (exclusive cumsum)
    out[b,t]= pred_c(t) + S_c(t) * y[b,t]

Strip-pipelined (strip = 64 timesteps = one 1 MiB SWDGE cast-load, 32 strips).
Per strip: 16 PE 128x128 transposes -> (t'',n)-on-partition layout, 4
matmuls [128,32]x[128,512] into a rotated 32-row PSUM band (rows are
chunk-major: m = 16c + 8j + 4q2 + t''), band->SBUF (ACT), 4 small [32->128]
PE transposes back to batch-major, one-op y-extract (ACT), chunk reduces
(DVE).  Per span (4 strips): scan cluster + affine (DVE), HWDGE store.
Span 7 drains chunk-incrementally to minimize the post-stream tail.

Engine discipline: DVE uses only tensor_tensor/reduce/scan (1-port ops, no
SWDGE descriptor stall); the big PSUM->SBUF copies are tensor_tensor with a
stride-1 bf16 zero operand so the DVE 2x packed mode stays eligible.
"""

import numpy as np

import concourse.bass as bass
import concourse.bacc as bacc
import concourse.tile as tile
from concourse import mybir
from concourse.bass_utils import run_bass_kernel_spmd
from concourse.masks import make_identity

F32 = mybir.dt.float32
BF16 = mybir.dt.bfloat16

B, T, NB = 1024, 2048, 32
NCORES = 8
BS = B // NCORES          # 128 batch rows per core = full partition dim
ADAPT = 32
C = T // ADAPT            # 64 chunks
STRIP_T = 64              # timesteps per strip = one 1 MiB f32 load
NSTRIP = T // STRIP_T     # 32
SF = STRIP_T * NB         # 2048 elements per partition per strip
PREFETCH = 14             # strip loads in flight ahead of compute
LAST_SPAN = NSTRIP - 4    # strips 28..31 drain chunk-incrementally

_cached_nc = None


def build_kernel():
    nc = bacc.Bacc("TRN2", target_bir_lowering=False, debug=False)

    x_ext = nc.dram_tensor("x", [BS, T * NB], F32, kind="ExternalInput")
    qb_ext = nc.dram_tensor("quant_bins", [NB, 1], F32, kind="ExternalInput")
    cs_ext = nc.dram_tensor("change_scales", [NB, 1], F32, kind="ExternalInput")
    out_ext = nc.dram_tensor("out", [BS, T], F32, kind="ExternalOutput")

    ADD = mybir.AluOpType.add
    MUL = mybir.AluOpType.mult
    BYP = mybir.AluOpType.bypass
    AFCOPY = mybir.ActivationFunctionType.Copy

    with tile.TileContext(nc) as tc:
        with (
            tc.tile_pool(name="consts", bufs=1) as consts,
            tc.tile_pool(name="xpool", bufs=16) as xpool,
            tc.tile_pool(name="xtpool", bufs=3) as xtpool,
            tc.tile_pool(name="ywpool", bufs=2) as ywpool,
            tc.tile_pool(name="ypool", bufs=2) as ypool,
            tc.tile_pool(name="accpool", bufs=1) as accpool,
            tc.tile_pool(name="ps_t", bufs=3, space="PSUM") as ps_t,
            tc.tile_pool(name="ps_b", bufs=2, space="PSUM") as ps_b,
            tc.tile_pool(name="ps_s", bufs=3, space="PSUM") as ps_s,
        ):
            # consts / scan chains (tiny DVE memsets: no Q7 stall)
            z1024 = consts.tile([128, 1024], BF16)
            nc.vector.memset(z1024[:], 0.0)
            inv32 = consts.tile([128, 1], F32)
            nc.vector.memset(inv32[:], 1.0 / ADAPT)
            S_chain = consts.tile([128, C + 1], F32)
            pred_chain = consts.tile([128, C + 1], F32)
            nc.vector.memset(S_chain[:, 0:1], 1.0)
            nc.vector.memset(pred_chain[:, 0:1], 0.0)
            m_buf = consts.tile([128, C], F32)
            p_buf = consts.tile([128, C], F32)
            tau_buf = consts.tile([128, C], F32)

            # qb/cs staging via HWDGE (keeps the SWDGE ring free)
            qbcs = consts.tile([128, 2], F32)
            for tp in range(4):
                nc.sync.dma_start(out=qbcs[32 * tp:32 * tp + 32, 0:1], in_=qb_ext[:])
                nc.sync.dma_start(out=qbcs[32 * tp:32 * tp + 32, 1:2], in_=cs_ext[:])

            xh = [
                xpool.tile([128, SF], BF16, name="xh", tag="xh")
                for _ in range(NSTRIP)
            ]

            def issue_load(s):
                nc.gpsimd.dma_start(out=xh[s][:], in_=x_ext[:, s * SF:(s + 1) * SF])

            issue_load(0)
            issue_load(1)

            ident_bf = consts.tile([128, 128], BF16)
            make_identity(nc, ident_bf[:])

            for s in range(2, PREFETCH):
                issue_load(s)

            # Stationary matrices A32_q [128, 32], q = 0..3 = 2*c + q2, on ACT.
            # Column m = 16*c + 8*j + 4*q2 + t'' (chunk-major row blocks);
            # A32_q[(t', n), m] = delta(t', t'') * (qb[n] if j==0 else cs[n]/32).
            A32 = []
            for q in range(4):
                c_, q2 = divmod(q, 2)
                Aq = consts.tile([128, 32], BF16, tag=f"A32_{q}")
                nc.scalar.memzero(Aq[:])
                for tp in range(4):
                    sl = slice(32 * tp, 32 * tp + 32)
                    m0 = 16 * c_ + 4 * q2 + tp
                    m1 = 16 * c_ + 8 + 4 * q2 + tp
                    nc.scalar.mul(Aq[sl, m0:m0 + 1], qbcs[sl, 0:1], 1.0)
                    nc.scalar.mul(Aq[sl, m1:m1 + 1], qbcs[sl, 1:2], 1.0 / ADAPT)
                A32.append(Aq)

            out_sb = accpool.tile([128, T], F32)

            bandof = {}   # s -> psum band tile
            ywof = {}     # s -> sbuf band tile
            slabof = {}   # s -> psum slab tile
            yspan = {}    # span k -> [128, 256] f32 y values

            def pe_strip_tr(sm):
                # [32,128] transposes of the yw band back to batch-major.
                # slab free index = 32*blk + 16*c + 8*j + 4*q2 + t''
                r = 32 * (sm % 4)
                yw = ywof.pop(sm)
                slab = ps_s.tile([128, 128], BF16)
                slabof[sm] = slab
                for blk2 in range(4):
                    nc.tensor.transpose(
                        slab[:, 32 * blk2:32 * blk2 + 32],
                        yw[r:r + 32, 128 * blk2:128 * (blk2 + 1)],
                        ident_bf[r:r + 32, r:r + 32],
                        tile_position=(r, 0),
                    )

            def act_band(sm):
                r = 32 * (sm % 4)
                band = bandof.pop(sm)
                yw = ywpool.tile([128, 512], BF16)
                ywof[sm] = yw
                nc.scalar.copy(out=yw[r:r + 32, :], in_=band[r:r + 32, :])

            def act_yext(sm):
                # one-op y extract; t(strip) = 32c + 16q2 + 4blk + t''
                k = sm // 4
                if sm % 4 == 0:
                    yspan[k] = ypool.tile([128, 4 * STRIP_T], F32, name="ysp", tag="ysp")
                slab = slabof[sm]
                sv = slab[:].rearrange(
                    "p (blk c j q2 t) -> p j c q2 blk t", blk=4, c=2, j=2, q2=2, t=4
                )
                for c in range(2):
                    dst = yspan[k][:, (sm % 4) * STRIP_T + 32 * c:
                                   (sm % 4) * STRIP_T + 32 * (c + 1)]
                    dv = dst.rearrange("p (q2 blk t) -> p q2 blk t", q2=2, blk=4, t=4)
                    nc.scalar.activation(
                        out=dv,
                        in_=sv[:, 0:1, c:c + 1].squeeze(1).squeeze(1),
                        func=AFCOPY,
                    )

            def dve_mred(sm):
                k = sm // 4
                ys = yspan[k][:, (sm % 4) * STRIP_T:(sm % 4 + 1) * STRIP_T]
                nc.vector.tensor_reduce(
                    out=m_buf[:, 2 * sm:2 * sm + 2],
                    in_=ys.rearrange("p (c u) -> p c u", c=2, u=ADAPT),
                    axis=mybir.AxisListType.X,
                    op=ADD,
                )

            def dve_wred(sm):
                slab = slabof.pop(sm)
                sv = slab[:].rearrange(
                    "p (blk c j q2 t) -> p j c q2 blk t", blk=4, c=2, j=2, q2=2, t=4
                )
                nc.vector.tensor_reduce(
                    out=p_buf[:, 2 * sm:2 * sm + 2],
                    in_=sv[:, 1:2].squeeze(1),
                    axis=mybir.AxisListType.XYZ,
                    op=ADD,
                )

            def dve_scans(c_lo, n, k_or_none=None):
                csl = slice(c_lo, c_lo + n)
                nc.vector.tensor_tensor_scan(
                    out=S_chain[:, c_lo + 1:c_lo + n + 1],
                    data0=p_buf[:, csl],
                    data1=p_buf[:, csl],
                    initial=S_chain[:, c_lo:c_lo + 1],
                    op0=MUL,
                    op1=BYP,
                )
                nc.vector.tensor_tensor(
                    out=tau_buf[:, csl], in0=S_chain[:, csl], in1=m_buf[:, csl], op=MUL
                )
                nc.vector.tensor_tensor(
                    out=tau_buf[:, csl], in0=tau_buf[:, csl],
                    in1=inv32[:, 0:1].broadcast_to([128, n]), op=MUL,
                )
                nc.vector.tensor_tensor_scan(
                    out=pred_chain[:, c_lo + 1:c_lo + n + 1],
                    data0=tau_buf[:, csl],
                    data1=tau_buf[:, csl],
                    initial=pred_chain[:, c_lo:c_lo + 1],
                    op0=ADD,
                    op1=BYP,
                )

            def dve_affine(c_lo, n, ysrc, y_off):
                # out = pred_c + S_c * y over n chunks
                csl = slice(c_lo, c_lo + n)
                o3 = out_sb[:, ADAPT * c_lo:ADAPT * (c_lo + n)].rearrange(
                    "p (c u) -> p c u", c=n, u=ADAPT
                )
                y3 = ysrc[:, y_off:y_off + n * ADAPT].rearrange(
                    "p (c u) -> p c u", c=n, u=ADAPT
                )
                S_bc = S_chain[:, csl].unsqueeze(2).broadcast_to([128, n, ADAPT])
                p_bc = pred_chain[:, csl].unsqueeze(2).broadcast_to([128, n, ADAPT])
                nc.vector.tensor_tensor(out=o3, in0=y3, in1=S_bc, op=MUL)
                nc.vector.tensor_tensor(out=o3, in0=o3, in1=p_bc, op=ADD)

            def sync_store(t_lo, t_hi):
                nc.sync.dma_start(out=out_ext[:, t_lo:t_hi], in_=out_sb[:, t_lo:t_hi])

            for s in range(NSTRIP):
                if s + PREFETCH < NSTRIP:
                    issue_load(s + PREFETCH)
                r = 32 * (s % 4)
                x_h = xh[s]

                # PE: 16 transposes -> (t'',n) on partitions, free = b
                xT = xtpool.tile([128, SF], BF16)
                for h in range(2):
                    pst = ps_t.tile([128, 1024], BF16)
                    for k in range(8):
                        blk = 8 * h + k
                        nc.tensor.transpose(
                            pst[:, k * 128:(k + 1) * 128],
                            x_h[:, blk * 128:(blk + 1) * 128],
                            ident_bf[:],
                        )
                    # DVE packed-2x copy as tensor_tensor (1-port; no Q7 stall)
                    nc.vector.tensor_tensor(
                        out=xT[:, h * 1024:(h + 1) * 1024], in0=pst[:],
                        in1=z1024[:], op=ADD,
                    )

                # PE: y/w projection into 32-row PSUM band at rotated position
                band = ps_b.tile([128, 512], F32)
                bandof[s] = band
                for q in range(4):
                    nc.tensor.matmul(
                        band[r:r + 32, :],
                        A32[q][:],
                        xT[:, q * 512:(q + 1) * 512],
                        start=(q == 0),
                        stop=(q == 3),
                        tile_position=(0, r),
                    )

                if s < LAST_SPAN:
                    # steady state: stages shifted to keep engines unblocked
                    if s >= 1:
                        act_band(s - 1)
                        pe_strip_tr(s - 1)
                        act_yext(s - 1)
                        dve_mred(s - 1)
                    if s >= 2:
                        dve_wred(s - 2)
                    if s >= 5 and s % 4 == 1:
                        k = (s - 5) // 4
                        dve_scans(8 * k, 8)
                        dve_affine(8 * k, 8, yspan.pop(k), 0)
                        sync_store(256 * k, 256 * (k + 1))
                elif s == LAST_SPAN:
                    # boundary: collapse the shifts, finish span 6 entirely
                    act_band(s - 1)
                    pe_strip_tr(s - 1)
                    act_yext(s - 1)
                    dve_mred(s - 1)
                    dve_wred(s - 2)
                    dve_wred(s - 1)
                    k = 6
                    dve_scans(8 * k, 8)
                    dve_affine(8 * k, 8, yspan.pop(k), 0)
                    sync_store(256 * k, 256 * (k + 1))
                if s >= LAST_SPAN:
                    # drain chunk-incrementally: no shifts, scan per strip
                    act_band(s)
                    pe_strip_tr(s)
                    act_yext(s)
                    dve_mred(s)
                    dve_wred(s)
                    dve_scans(2 * s, 2)
                    dve_affine(2 * s, 2, yspan[7], (s % 4) * STRIP_T)
                    sync_store(STRIP_T * s, STRIP_T * (s + 1))
            yspan.pop(7)

    nc.compile()
    return nc


def make_in_maps(inputs):
    x = np.ascontiguousarray(inputs["x"], dtype=np.float32)
    qb = np.ascontiguousarray(inputs["quant_bins"], dtype=np.float32).reshape(NB, 1)
    cs = np.ascontiguousarray(inputs["change_scales"], dtype=np.float32).reshape(NB, 1)
    return [
        {
            "x": x[i * BS:(i + 1) * BS].reshape(BS, T * NB),
            "quant_bins": qb,
            "change_scales": cs,
        }
        for i in range(NCORES)
    ]


def gather_out(res):
    out = np.concatenate([res.results[i]["out"] for i in range(NCORES)], axis=0)
    return out.astype(np.float32)


def kernel(x, quant_bins, change_scales):
    global _cached_nc
    if _cached_nc is None:
        _cached_nc = build_kernel()
    nc = _cached_nc

    in_maps = make_in_maps(
        {"x": x, "quant_bins": quant_bins, "change_scales": change_scales}
    )
    res = run_bass_kernel_spmd(nc, in_maps, core_ids=list(range(NCORES)))
    return gather_out(res)


if __name__ == "__main__":
    rng = np.random.default_rng(0)
    x = rng.standard_normal((B, T, NB)).astype(np.float32)
    qb = rng.standard_normal((NB,)).astype(np.float32)
    cs = rng.uniform(0.9, 1.1, (NB, 1)).astype(np.float32)
    out = kernel(x=x, quant_bins=qb, change_scales=cs)
    print("out", out.shape, out.dtype)


# revision 10
# speedup vs baseline: 1.1326x; 1.1272x over previous
"""DeltaDequantization Trainium2 kernel (8-core SPMD, pure data parallel over batch).

Math (per batch element b, chunks c of 32 steps):
    scale_c = (1/32) * sum_{s,n} x[b,c,s,n] * cs[n]          (independent of carry!)
    S_c     = prod_{c'<c} scale_c'          (exclusive cumprod)
    y[b,t]  = sum_n x[b,t,n] * qb[n]
    m_c     = (1/32) * sum_{s in c} y[b,t]
    pred_c  = sum_{c'<c} S_c' * m_c'        (exclusive cumsum)
    out[b,t]= pred_c(t) + S_c(t) * y[b,t]

Strip-pipelined (strip = 64 timesteps = one 1 MiB SWDGE cast-load, 32 strips).
Per strip: 16 PE 128x128 transposes -> (t'',n)-on-partition layout, 4
matmuls [128,32]x[128,512] into a rotated 32-row PSUM band (rows are
chunk-major: m = 16c + 8j + 4q2 + t''), band->SBUF (ACT), 4 small [32->128]
PE transposes back to batch-major, one-op y-extract (ACT), chunk reduces
(DVE).  Per span (4 strips): scan cluster + affine (DVE), HWDGE store.
Span 7 drains chunk-incrementally to minimize the post-stream tail.

Engine discipline: DVE uses only tensor_tensor/reduce/scan (1-port ops, no
SWDGE descriptor stall); the big PSUM->SBUF copies are tensor_tensor with a
stride-1 bf16 zero operand so the DVE 2x packed mode stays eligible.
"""

import numpy as np

import concourse.bass as bass
import concourse.bacc as bacc
import concourse.tile as tile
from concourse import mybir
from concourse.bass_utils import run_bass_kernel_spmd
from concourse.masks import make_identity

F32 = mybir.dt.float32
BF16 = mybir.dt.bfloat16

B, T, NB = 1024, 2048, 32
NCORES = 8
BS = B // NCORES          # 128 batch rows per core = full partition dim
ADAPT = 32
C = T // ADAPT            # 64 chunks
STRIP_T = 64              # timesteps per strip = one 1 MiB f32 load
NSTRIP = T // STRIP_T     # 32
SF = STRIP_T * NB         # 2048 elements per partition per strip
PREFETCH = 14             # strip loads in flight ahead of compute
LAST_SPAN = NSTRIP - 4    # strips 28..31 drain chunk-incrementally

_cached_nc = None


def build_kernel():
    nc = bacc.Bacc("TRN2", target_bir_lowering=False, debug=False)

    x_ext = nc.dram_tensor("x", [BS, T * NB], F32, kind="ExternalInput")
    qb_ext = nc.dram_tensor("quant_bins", [NB, 1], F32, kind="ExternalInput")
    cs_ext = nc.dram_tensor("change_scales", [NB, 1], F32, kind="ExternalInput")
    out_ext = nc.dram_tensor("out", [BS, T], F32, kind="ExternalOutput")

    ADD = mybir.AluOpType.add
    MUL = mybir.AluOpType.mult
    BYP = mybir.AluOpType.bypass
    AFCOPY = mybir.ActivationFunctionType.Copy

    with tile.TileContext(nc) as tc:
        with (
            tc.tile_pool(name="consts", bufs=1) as consts,
            tc.tile_pool(name="xpool", bufs=16) as xpool,
            tc.tile_pool(name="xtpool", bufs=3) as xtpool,
            tc.tile_pool(name="ywpool", bufs=2) as ywpool,
            tc.tile_pool(name="ypool", bufs=2) as ypool,
            tc.tile_pool(name="accpool", bufs=1) as accpool,
            tc.tile_pool(name="ps_t", bufs=3, space="PSUM") as ps_t,
            tc.tile_pool(name="ps_b", bufs=2, space="PSUM") as ps_b,
            tc.tile_pool(name="ps_s", bufs=3, space="PSUM") as ps_s,
        ):
            # consts / scan chains (tiny DVE memsets: no Q7 stall)
            z1024 = consts.tile([128, 1024], BF16)
            nc.vector.memset(z1024[:], 0.0)
            inv32 = consts.tile([128, 1], F32)
            nc.vector.memset(inv32[:], 1.0 / ADAPT)
            S_chain = consts.tile([128, C + 1], F32)
            pred_chain = consts.tile([128, C + 1], F32)
            nc.vector.memset(S_chain[:, 0:1], 1.0)
            nc.vector.memset(pred_chain[:, 0:1], 0.0)
            m_buf = consts.tile([128, C], F32)
            p_buf = consts.tile([128, C], F32)
            tau_buf = consts.tile([128, C], F32)

            # qb/cs staging via HWDGE (keeps the SWDGE ring free)
            qbcs = consts.tile([128, 2], F32)
            for tp in range(4):
                nc.sync.dma_start(out=qbcs[32 * tp:32 * tp + 32, 0:1], in_=qb_ext[:])
                nc.sync.dma_start(out=qbcs[32 * tp:32 * tp + 32, 1:2], in_=cs_ext[:])

            xh = [
                xpool.tile([128, SF], BF16, name="xh", tag="xh")
                for _ in range(NSTRIP)
            ]

            def issue_load(s):
                nc.gpsimd.dma_start(out=xh[s][:], in_=x_ext[:, s * SF:(s + 1) * SF])

            issue_load(0)
            issue_load(1)

            ident_bf = consts.tile([128, 128], BF16)
            make_identity(nc, ident_bf[:])

            for s in range(2, PREFETCH):
                issue_load(s)

            # Stationary matrices A32_q [128, 32], q = 0..3 = 2*c + q2, on ACT.
            # Column m = 16*c + 8*j + 4*q2 + t'' (chunk-major row blocks);
            # A32_q[(t', n), m] = delta(t', t'') * (qb[n] if j==0 else cs[n]/32).
            A32 = []
            for q in range(4):
                c_, q2 = divmod(q, 2)
                Aq = consts.tile([128, 32], BF16, tag=f"A32_{q}")
                nc.scalar.memzero(Aq[:])
                for tp in range(4):
                    sl = slice(32 * tp, 32 * tp + 32)
                    m0 = 16 * c_ + 4 * q2 + tp
                    m1 = 16 * c_ + 8 + 4 * q2 + tp
                    nc.scalar.mul(Aq[sl, m0:m0 + 1], qbcs[sl, 0:1], 1.0)
                    nc.scalar.mul(Aq[sl, m1:m1 + 1], qbcs[sl, 1:2], 1.0 / ADAPT)
                A32.append(Aq)

            out_sb = accpool.tile([128, T], F32)

            bandof = {}   # s -> psum band tile
            ywof = {}     # s -> sbuf band tile
            slabof = {}   # s -> psum slab tile
            yspan = {}    # span k -> [128, 256] f32 y values

            def pe_strip_tr(sm):
                # [32,128] transposes of the yw band back to batch-major.
                # slab free index = 32*blk + 16*c + 8*j + 4*q2 + t''
                r = 32 * (sm % 4)
                yw = ywof.pop(sm)
                slab = ps_s.tile([128, 128], BF16)
                slabof[sm] = slab
                for blk2 in range(4):
                    nc.tensor.transpose(
                        slab[:, 32 * blk2:32 * blk2 + 32],
                        yw[r:r + 32, 128 * blk2:128 * (blk2 + 1)],
                        ident_bf[r:r + 32, r:r + 32],
                        tile_position=(r, 0),
                    )

            def act_band(sm):
                r = 32 * (sm % 4)
                band = bandof.pop(sm)
                yw = ywpool.tile([128, 512], BF16)
                ywof[sm] = yw
                nc.scalar.copy(out=yw[r:r + 32, :], in_=band[r:r + 32, :])

            def act_yext(sm):
                # one-op y extract; t(strip) = 32c + 16q2 + 4blk + t''
                k = sm // 4
                if sm % 4 == 0:
                    yspan[k] = ypool.tile([128, 4 * STRIP_T], F32, name="ysp", tag="ysp")
                slab = slabof[sm]
                sv = slab[:].rearrange(
                    "p (blk c j q2 t) -> p j c q2 blk t", blk=4, c=2, j=2, q2=2, t=4
                )
                for c in range(2):
                    dst = yspan[k][:, (sm % 4) * STRIP_T + 32 * c:
                                   (sm % 4) * STRIP_T + 32 * (c + 1)]
                    dv = dst.rearrange("p (q2 blk t) -> p q2 blk t", q2=2, blk=4, t=4)
                    nc.scalar.activation(
                        out=dv,
                        in_=sv[:, 0:1, c:c + 1].squeeze(1).squeeze(1),
                        func=AFCOPY,
                    )

            def dve_mred(sm):
                k = sm // 4
                ys = yspan[k][:, (sm % 4) * STRIP_T:(sm % 4 + 1) * STRIP_T]
                nc.vector.tensor_reduce(
                    out=m_buf[:, 2 * sm:2 * sm + 2],
                    in_=ys.rearrange("p (c u) -> p c u", c=2, u=ADAPT),
                    axis=mybir.AxisListType.X,
                    op=ADD,
                )

            def dve_wred(sm):
                slab = slabof.pop(sm)
                sv = slab[:].rearrange(
                    "p (blk c j q2 t) -> p j c q2 blk t", blk=4, c=2, j=2, q2=2, t=4
                )
                nc.vector.tensor_reduce(
                    out=p_buf[:, 2 * sm:2 * sm + 2],
                    in_=sv[:, 1:2].squeeze(1),
                    axis=mybir.AxisListType.XYZ,
                    op=ADD,
                )

            def dve_scans(c_lo, n, k_or_none=None):
                csl = slice(c_lo, c_lo + n)
                nc.vector.tensor_tensor_scan(
                    out=S_chain[:, c_lo + 1:c_lo + n + 1],
                    data0=p_buf[:, csl],
                    data1=p_buf[:, csl],
                    initial=S_chain[:, c_lo:c_lo + 1],
                    op0=MUL,
                    op1=BYP,
                )
                nc.vector.tensor_tensor(
                    out=tau_buf[:, csl], in0=S_chain[:, csl], in1=m_buf[:, csl], op=MUL
                )
                nc.vector.tensor_tensor(
                    out=tau_buf[:, csl], in0=tau_buf[:, csl],
                    in1=inv32[:, 0:1].broadcast_to([128, n]), op=MUL,
                )
                nc.vector.tensor_tensor_scan(
                    out=pred_chain[:, c_lo + 1:c_lo + n + 1],
                    data0=tau_buf[:, csl],
                    data1=tau_buf[:, csl],
                    initial=pred_chain[:, c_lo:c_lo + 1],
                    op0=ADD,
                    op1=BYP,
                )

            def dve_affine(c_lo, n, ysrc, y_off):
                # out = pred_c + S_c * y over n chunks
                csl = slice(c_lo, c_lo + n)
                o3 = out_sb[:, ADAPT * c_lo:ADAPT * (c_lo + n)].rearrange(
                    "p (c u) -> p c u", c=n, u=ADAPT
                )
                y3 = ysrc[:, y_off:y_off + n * ADAPT].rearrange(
                    "p (c u) -> p c u", c=n, u=ADAPT
                )
                S_bc = S_chain[:, csl].unsqueeze(2).broadcast_to([128, n, ADAPT])
                p_bc = pred_chain[:, csl].unsqueeze(2).broadcast_to([128, n, ADAPT])
                nc.vector.tensor_tensor(out=o3, in0=y3, in1=S_bc, op=MUL)
                nc.vector.tensor_tensor(out=o3, in0=o3, in1=p_bc, op=ADD)

            def sync_store(t_lo, t_hi):
                nc.sync.dma_start(out=out_ext[:, t_lo:t_hi], in_=out_sb[:, t_lo:t_hi])

            xtof = {}     # s -> xT tile

            def pe_tr(s):
                # PE: 16 transposes -> (t'',n) on partitions, free = b
                # DVE drains each pst half with a packed-2x tensor_tensor
                x_h = xh[s]
                xT = xtpool.tile([128, SF], BF16)
                xtof[s] = xT
                for h in range(2):
                    pst = ps_t.tile([128, 1024], BF16)
                    for k in range(8):
                        blk = 8 * h + k
                        nc.tensor.transpose(
                            pst[:, k * 128:(k + 1) * 128],
                            x_h[:, blk * 128:(blk + 1) * 128],
                            ident_bf[:],
                        )
                    nc.vector.tensor_tensor(
                        out=xT[:, h * 1024:(h + 1) * 1024], in0=pst[:],
                        in1=z1024[:], op=ADD,
                    )

            def pe_mm(s):
                # PE: y/w projection into 32-row PSUM band at rotated position
                r = 32 * (s % 4)
                xT = xtof.pop(s)
                band = ps_b.tile([128, 512], F32)
                bandof[s] = band
                for q in range(4):
                    nc.tensor.matmul(
                        band[r:r + 32, :],
                        A32[q][:],
                        xT[:, q * 512:(q + 1) * 512],
                        start=(q == 0),
                        stop=(q == 3),
                        tile_position=(0, r),
                    )

            def finish_strip(sm):
                # band -> SBUF -> slab -> y/m/p (immediate, for the drain)
                act_band(sm)
                pe_strip_tr(sm)
                act_yext(sm)
                dve_mred(sm)

            for s in range(NSTRIP):
                if s + PREFETCH < NSTRIP:
                    issue_load(s + PREFETCH)

                if s < LAST_SPAN:
                    # steady state: stages shifted so no engine blocks in-order
                    pe_tr(s)
                    if s >= 2:
                        act_band(s - 2)
                        pe_strip_tr(s - 2)
                    if s >= 1:
                        pe_mm(s - 1)
                    if s >= 2:
                        act_yext(s - 2)
                        dve_mred(s - 2)
                    if s >= 3:
                        dve_wred(s - 3)
                    if s >= 6 and s % 4 == 2:
                        k = (s - 6) // 4
                        dve_scans(8 * k, 8)
                        dve_affine(8 * k, 8, yspan.pop(k), 0)
                        sync_store(256 * k, 256 * (k + 1))
                else:
                    if s == LAST_SPAN:
                        # collapse the shifts: finish strips 25..27 and span 6
                        pe_mm(s - 1)
                        act_band(s - 2)
                        pe_strip_tr(s - 2)
                        act_yext(s - 2)
                        dve_mred(s - 2)
                        dve_wred(s - 3)
                        finish_strip(s - 1)
                        dve_wred(s - 2)
                        dve_wred(s - 1)
                        k = 6
                        dve_scans(8 * k, 8)
                        dve_affine(8 * k, 8, yspan.pop(k), 0)
                        sync_store(256 * k, 256 * (k + 1))
                    # drain chunk-incrementally: no shifts, scan per strip
                    pe_tr(s)
                    pe_mm(s)
                    finish_strip(s)
                    dve_wred(s)
                    dve_scans(2 * s, 2)
                    dve_affine(2 * s, 2, yspan[7], (s % 4) * STRIP_T)
                    sync_store(STRIP_T * s, STRIP_T * (s + 1))
            yspan.pop(7)

    nc.compile()
    return nc


def make_in_maps(inputs):
    x = np.ascontiguousarray(inputs["x"], dtype=np.float32)
    qb = np.ascontiguousarray(inputs["quant_bins"], dtype=np.float32).reshape(NB, 1)
    cs = np.ascontiguousarray(inputs["change_scales"], dtype=np.float32).reshape(NB, 1)
    return [
        {
            "x": x[i * BS:(i + 1) * BS].reshape(BS, T * NB),
            "quant_bins": qb,
            "change_scales": cs,
        }
        for i in range(NCORES)
    ]


def gather_out(res):
    out = np.concatenate([res.results[i]["out"] for i in range(NCORES)], axis=0)
    return out.astype(np.float32)


def kernel(x, quant_bins, change_scales):
    global _cached_nc
    if _cached_nc is None:
        _cached_nc = build_kernel()
    nc = _cached_nc

    in_maps = make_in_maps(
        {"x": x, "quant_bins": quant_bins, "change_scales": change_scales}
    )
    res = run_bass_kernel_spmd(nc, in_maps, core_ids=list(range(NCORES)))
    return gather_out(res)


if __name__ == "__main__":
    rng = np.random.default_rng(0)
    x = rng.standard_normal((B, T, NB)).astype(np.float32)
    qb = rng.standard_normal((NB,)).astype(np.float32)
    cs = rng.uniform(0.9, 1.1, (NB, 1)).astype(np.float32)
    out = kernel(x=x, quant_bins=qb, change_scales=cs)
    print("out", out.shape, out.dtype)


# revision 11
# speedup vs baseline: 1.1369x; 1.0038x over previous
"""DeltaDequantization Trainium2 kernel (8-core SPMD, pure data parallel over batch).

Math (per batch element b, chunks c of 32 steps):
    scale_c = (1/32) * sum_{s,n} x[b,c,s,n] * cs[n]          (independent of carry!)
    S_c     = prod_{c'<c} scale_c'          (exclusive cumprod)
    y[b,t]  = sum_n x[b,t,n] * qb[n]
    m_c     = (1/32) * sum_{s in c} y[b,t]
    pred_c  = sum_{c'<c} S_c' * m_c'        (exclusive cumsum)
    out[b,t]= pred_c(t) + S_c(t) * y[b,t]

Strip-pipelined (strip = 64 timesteps = one 1 MiB SWDGE cast-load, 32 strips).
Per strip: 16 PE 128x128 transposes -> (t'',n)-on-partition layout, 4
matmuls [128,32]x[128,512] into a rotated 32-row PSUM band (rows are
chunk-major: m = 16c + 8j + 4q2 + t''), band->SBUF (ACT), 4 small [32->128]
PE transposes back to batch-major, one-op y-extract (ACT), chunk reduces
(DVE).  Per span (4 strips): scan cluster + affine (DVE), HWDGE store.
Span 7 drains chunk-incrementally to minimize the post-stream tail.

Engine discipline: DVE uses only tensor_tensor/reduce/scan (1-port ops, no
SWDGE descriptor stall); the big PSUM->SBUF copies are tensor_tensor with a
stride-1 bf16 zero operand so the DVE 2x packed mode stays eligible.
"""

import numpy as np

import concourse.bass as bass
import concourse.bacc as bacc
import concourse.tile as tile
from concourse import mybir
from concourse.bass_utils import run_bass_kernel_spmd
from concourse.masks import make_identity

F32 = mybir.dt.float32
BF16 = mybir.dt.bfloat16

B, T, NB = 1024, 2048, 32
NCORES = 8
BS = B // NCORES          # 128 batch rows per core = full partition dim
ADAPT = 32
C = T // ADAPT            # 64 chunks
STRIP_T = 64              # timesteps per strip = one 1 MiB f32 load
NSTRIP = T // STRIP_T     # 32
SF = STRIP_T * NB         # 2048 elements per partition per strip
PREFETCH = 14             # strip loads in flight ahead of compute
LAST_SPAN = NSTRIP - 4    # strips 28..31 drain chunk-incrementally

_cached_nc = None


def build_kernel():
    nc = bacc.Bacc("TRN2", target_bir_lowering=False, debug=False)

    x_ext = nc.dram_tensor("x", [BS, T * NB], F32, kind="ExternalInput")
    qb_ext = nc.dram_tensor("quant_bins", [NB, 1], F32, kind="ExternalInput")
    cs_ext = nc.dram_tensor("change_scales", [NB, 1], F32, kind="ExternalInput")
    out_ext = nc.dram_tensor("out", [BS, T], F32, kind="ExternalOutput")

    ADD = mybir.AluOpType.add
    MUL = mybir.AluOpType.mult
    BYP = mybir.AluOpType.bypass
    AFCOPY = mybir.ActivationFunctionType.Copy

    with tile.TileContext(nc) as tc:
        with (
            tc.tile_pool(name="consts", bufs=1) as consts,
            tc.tile_pool(name="xpool", bufs=16) as xpool,
            tc.tile_pool(name="xtpool", bufs=3) as xtpool,
            tc.tile_pool(name="ywpool", bufs=2) as ywpool,
            tc.tile_pool(name="ypool", bufs=2) as ypool,
            tc.tile_pool(name="accpool", bufs=1) as accpool,
            tc.tile_pool(name="ps_t", bufs=3, space="PSUM") as ps_t,
            tc.tile_pool(name="ps_b", bufs=2, space="PSUM") as ps_b,
            tc.tile_pool(name="ps_s", bufs=3, space="PSUM") as ps_s,
        ):
            # consts / scan chains (tiny DVE memsets: no Q7 stall)
            z1024 = consts.tile([128, 1024], BF16)
            nc.vector.memset(z1024[:], 0.0)
            inv32 = consts.tile([128, 1], F32)
            nc.vector.memset(inv32[:], 1.0 / ADAPT)
            S_chain = consts.tile([128, C + 1], F32)
            pred_chain = consts.tile([128, C + 1], F32)
            nc.vector.memset(S_chain[:, 0:1], 1.0)
            nc.vector.memset(pred_chain[:, 0:1], 0.0)
            m_buf = consts.tile([128, C], F32)
            p_buf = consts.tile([128, C], F32)
            tau_buf = consts.tile([128, C], F32)

            # qb/cs staging via HWDGE (keeps the SWDGE ring free)
            qbcs = consts.tile([128, 2], F32)
            for tp in range(4):
                nc.sync.dma_start(out=qbcs[32 * tp:32 * tp + 32, 0:1], in_=qb_ext[:])
                nc.sync.dma_start(out=qbcs[32 * tp:32 * tp + 32, 1:2], in_=cs_ext[:])

            xh = [
                xpool.tile([128, SF], BF16, name="xh", tag="xh")
                for _ in range(NSTRIP)
            ]

            def issue_load(s):
                nc.gpsimd.dma_start(out=xh[s][:], in_=x_ext[:, s * SF:(s + 1) * SF])

            issue_load(0)
            issue_load(1)

            ident_bf = consts.tile([128, 128], BF16)
            make_identity(nc, ident_bf[:])

            for s in range(2, PREFETCH):
                issue_load(s)

            # Stationary matrices A32_q [128, 32], q = 0..3 = 2*c + q2, on ACT.
            # Column m = 16*c + 8*j + 4*q2 + t'' (chunk-major row blocks);
            # A32_q[(t', n), m] = delta(t', t'') * (qb[n] if j==0 else cs[n]/32).
            A32 = []
            for q in range(4):
                c_, q2 = divmod(q, 2)
                Aq = consts.tile([128, 32], BF16, tag=f"A32_{q}")
                nc.scalar.memzero(Aq[:])
                for tp in range(4):
                    sl = slice(32 * tp, 32 * tp + 32)
                    m0 = 16 * c_ + 4 * q2 + tp
                    m1 = 16 * c_ + 8 + 4 * q2 + tp
                    nc.scalar.mul(Aq[sl, m0:m0 + 1], qbcs[sl, 0:1], 1.0)
                    nc.scalar.mul(Aq[sl, m1:m1 + 1], qbcs[sl, 1:2], 1.0 / ADAPT)
                A32.append(Aq)

            out_sb = accpool.tile([128, T], F32)

            bandof = {}   # s -> psum band tile
            ywof = {}     # s -> sbuf band tile
            slabof = {}   # s -> psum slab tile
            yspan = {}    # span k -> [128, 256] f32 y values

            def pe_strip_tr(sm):
                # [32,128] transposes of the yw band back to batch-major.
                # slab free index = 32*blk + 16*c + 8*j + 4*q2 + t''
                r = 32 * (sm % 4)
                yw = ywof.pop(sm)
                slab = ps_s.tile([128, 128], BF16)
                slabof[sm] = slab
                for blk2 in range(4):
                    nc.tensor.transpose(
                        slab[:, 32 * blk2:32 * blk2 + 32],
                        yw[r:r + 32, 128 * blk2:128 * (blk2 + 1)],
                        ident_bf[r:r + 32, r:r + 32],
                        tile_position=(r, 0),
                    )

            def act_band(sm):
                r = 32 * (sm % 4)
                band = bandof.pop(sm)
                yw = ywpool.tile([128, 512], BF16)
                ywof[sm] = yw
                nc.scalar.copy(out=yw[r:r + 32, :], in_=band[r:r + 32, :])

            def act_yext(sm):
                # one-op y extract; t(strip) = 32c + 16q2 + 4blk + t''
                k = sm // 4
                if sm % 4 == 0:
                    yspan[k] = ypool.tile([128, 4 * STRIP_T], F32, name="ysp", tag="ysp")
                slab = slabof[sm]
                sv = slab[:].rearrange(
                    "p (blk c j q2 t) -> p j c q2 blk t", blk=4, c=2, j=2, q2=2, t=4
                )
                for c in range(2):
                    dst = yspan[k][:, (sm % 4) * STRIP_T + 32 * c:
                                   (sm % 4) * STRIP_T + 32 * (c + 1)]
                    dv = dst.rearrange("p (q2 blk t) -> p q2 blk t", q2=2, blk=4, t=4)
                    nc.scalar.activation(
                        out=dv,
                        in_=sv[:, 0:1, c:c + 1].squeeze(1).squeeze(1),
                        func=AFCOPY,
                    )

            def dve_mred(sm):
                k = sm // 4
                ys = yspan[k][:, (sm % 4) * STRIP_T:(sm % 4 + 1) * STRIP_T]
                nc.vector.tensor_reduce(
                    out=m_buf[:, 2 * sm:2 * sm + 2],
                    in_=ys.rearrange("p (c u) -> p c u", c=2, u=ADAPT),
                    axis=mybir.AxisListType.X,
                    op=ADD,
                )

            def dve_wred(sm):
                slab = slabof.pop(sm)
                sv = slab[:].rearrange(
                    "p (blk c j q2 t) -> p j c q2 blk t", blk=4, c=2, j=2, q2=2, t=4
                )
                nc.vector.tensor_reduce(
                    out=p_buf[:, 2 * sm:2 * sm + 2],
                    in_=sv[:, 1:2].squeeze(1),
                    axis=mybir.AxisListType.XYZ,
                    op=ADD,
                )

            def dve_scans(c_lo, n, k_or_none=None):
                csl = slice(c_lo, c_lo + n)
                nc.vector.tensor_tensor_scan(
                    out=S_chain[:, c_lo + 1:c_lo + n + 1],
                    data0=p_buf[:, csl],
                    data1=p_buf[:, csl],
                    initial=S_chain[:, c_lo:c_lo + 1],
                    op0=MUL,
                    op1=BYP,
                )
                nc.vector.tensor_tensor(
                    out=tau_buf[:, csl], in0=S_chain[:, csl], in1=m_buf[:, csl], op=MUL
                )
                nc.vector.tensor_tensor(
                    out=tau_buf[:, csl], in0=tau_buf[:, csl],
                    in1=inv32[:, 0:1].broadcast_to([128, n]), op=MUL,
                )
                nc.vector.tensor_tensor_scan(
                    out=pred_chain[:, c_lo + 1:c_lo + n + 1],
                    data0=tau_buf[:, csl],
                    data1=tau_buf[:, csl],
                    initial=pred_chain[:, c_lo:c_lo + 1],
                    op0=ADD,
                    op1=BYP,
                )

            def dve_affine(c_lo, n, ysrc, y_off):
                # out = pred_c + S_c * y over n chunks
                csl = slice(c_lo, c_lo + n)
                o3 = out_sb[:, ADAPT * c_lo:ADAPT * (c_lo + n)].rearrange(
                    "p (c u) -> p c u", c=n, u=ADAPT
                )
                y3 = ysrc[:, y_off:y_off + n * ADAPT].rearrange(
                    "p (c u) -> p c u", c=n, u=ADAPT
                )
                S_bc = S_chain[:, csl].unsqueeze(2).broadcast_to([128, n, ADAPT])
                p_bc = pred_chain[:, csl].unsqueeze(2).broadcast_to([128, n, ADAPT])
                nc.vector.tensor_tensor(out=o3, in0=y3, in1=S_bc, op=MUL)
                nc.vector.tensor_tensor(out=o3, in0=o3, in1=p_bc, op=ADD)

            def sync_store(t_lo, t_hi):
                nc.sync.dma_start(out=out_ext[:, t_lo:t_hi], in_=out_sb[:, t_lo:t_hi])

            xtof = {}     # s -> xT tile

            def pe_tr(s):
                # PE: 16 transposes -> (t'',n) on partitions, free = b
                # DVE drains each pst half with a packed-2x tensor_tensor
                x_h = xh[s]
                xT = xtpool.tile([128, SF], BF16)
                xtof[s] = xT
                for h in range(2):
                    pst = ps_t.tile([128, 1024], BF16)
                    for k in range(8):
                        blk = 8 * h + k
                        nc.tensor.transpose(
                            pst[:, k * 128:(k + 1) * 128],
                            x_h[:, blk * 128:(blk + 1) * 128],
                            ident_bf[:],
                        )
                    nc.vector.tensor_tensor(
                        out=xT[:, h * 1024:(h + 1) * 1024], in0=pst[:],
                        in1=z1024[:], op=ADD,
                    )

            def pe_mm(s):
                # PE: y/w projection into 32-row PSUM band at rotated position
                r = 32 * (s % 4)
                xT = xtof.pop(s)
                band = ps_b.tile([128, 512], F32)
                bandof[s] = band
                for q in range(4):
                    nc.tensor.matmul(
                        band[r:r + 32, :],
                        A32[q][:],
                        xT[:, q * 512:(q + 1) * 512],
                        start=(q == 0),
                        stop=(q == 3),
                        tile_position=(0, r),
                    )

            def finish_strip(sm):
                # band -> SBUF -> slab -> y/m/p (immediate, for the drain)
                act_band(sm)
                pe_strip_tr(sm)
                act_yext(sm)
                dve_mred(sm)

            for s in range(NSTRIP):
                if s + PREFETCH < NSTRIP:
                    issue_load(s + PREFETCH)

                # steady state: stages shifted so no engine blocks in-order
                pe_tr(s)
                if s >= 2:
                    act_band(s - 2)
                    pe_strip_tr(s - 2)
                if s >= 1:
                    pe_mm(s - 1)
                if s >= 2:
                    act_yext(s - 2)
                    dve_mred(s - 2)
                if s >= 3:
                    dve_wred(s - 3)
                if s >= 6 and s % 4 == 2:
                    k = (s - 6) // 4
                    dve_scans(8 * k, 8)
                    dve_affine(8 * k, 8, yspan.pop(k), 0)

            # drain the shifted pipeline; all stores deferred out of the
            # stream window (in-stream stores cost ~9us of SDMA engine time
            # and HBM read/write turnaround)
            pe_mm(NSTRIP - 1)
            for sm in (NSTRIP - 2, NSTRIP - 1):
                finish_strip(sm)
            dve_wred(NSTRIP - 3)
            dve_wred(NSTRIP - 2)
            sync_store(0, 256 * 7)          # spans 0-6: hidden under the drain
            dve_wred(NSTRIP - 1)
            dve_scans(8 * 7, 8)
            dve_affine(8 * 7, 8, yspan.pop(7), 0)
            sync_store(256 * 7, 256 * 8)    # last span

    nc.compile()
    return nc


def make_in_maps(inputs):
    x = np.ascontiguousarray(inputs["x"], dtype=np.float32)
    qb = np.ascontiguousarray(inputs["quant_bins"], dtype=np.float32).reshape(NB, 1)
    cs = np.ascontiguousarray(inputs["change_scales"], dtype=np.float32).reshape(NB, 1)
    return [
        {
            "x": x[i * BS:(i + 1) * BS].reshape(BS, T * NB),
            "quant_bins": qb,
            "change_scales": cs,
        }
        for i in range(NCORES)
    ]


def gather_out(res):
    out = np.concatenate([res.results[i]["out"] for i in range(NCORES)], axis=0)
    return out.astype(np.float32)


def kernel(x, quant_bins, change_scales):
    global _cached_nc
    if _cached_nc is None:
        _cached_nc = build_kernel()
    nc = _cached_nc

    in_maps = make_in_maps(
        {"x": x, "quant_bins": quant_bins, "change_scales": change_scales}
    )
    res = run_bass_kernel_spmd(nc, in_maps, core_ids=list(range(NCORES)))
    return gather_out(res)


if __name__ == "__main__":
    rng = np.random.default_rng(0)
    x = rng.standard_normal((B, T, NB)).astype(np.float32)
    qb = rng.standard_normal((NB,)).astype(np.float32)
    cs = rng.uniform(0.9, 1.1, (NB, 1)).astype(np.float32)
    out = kernel(x=x, quant_bins=qb, change_scales=cs)
    print("out", out.shape, out.dtype)
